# revision 1
# baseline (speedup 1.0000x reference)
"""Causal self-attention (B=4, T=2048, D=1024, H=16) on 8 TRN2 NeuronCores.

Sharding: data parallel over batch (4 batches x 2 core-pairs) and tensor
parallel over heads (8 heads per core). Each core:
  - projects its batch's tokens to Q/K/V for its 8 heads (fp16 matmuls,
    fp32 PSUM accumulation),
  - runs causal flash-style attention in "scores transposed" orientation
    (S_T[key, query] = K_feat.T-stationary @ Q_feat-moving) so the softmax
    probabilities come out in the right orientation to be the stationary
    operand of P@V with no transpose,
  - softmax without max-subtraction (scores ~ N(0,1); fp32 exp range is
    ample) with denominators from an extra ones-column appended to V,
  - pairwise AllGather exchanges attention outputs between the two cores
    of a batch, then each core computes the final projection for its half
    of the tokens.
Host reassembles the full (4, 2048, 1024) output.
"""

import numpy as np

import concourse.bass as bass
import concourse.mybir as mybir
import concourse.tile as tile
from concourse import bacc, bass_utils
from concourse.bass import ds

N_CORES = 8
B, T, D, H = 4, 2048, 1024, 16
HD = D // H  # 64
FH = 512  # features per core (8 heads)
NFG = 4  # feature groups of 128 (2 heads each) per core
NTCH = 4  # 512-token chunks
NDS = 8  # 128-row contraction sub-tiles of D
NQC = 4  # 512-query chunks
NTT = 16  # 128-token tiles
F16 = mybir.dt.float16
F32 = mybir.dt.float32
EXP_SCALE = float(1.0 / np.sqrt(HD))
EXP_SCALE_DUP = EXP_SCALE / 2.0


def build_nc(sim_mode=False, phase="full"):
    nc = bacc.Bacc("TRN2", target_bir_lowering=False, debug=False, num_devices=N_CORES)

    xT_d = nc.dram_tensor("xT", (D, T), F16, kind="ExternalInput")
    wq_d = nc.dram_tensor("wq", (D, FH), F16, kind="ExternalInput")
    wk_d = nc.dram_tensor("wk", (D, FH), F16, kind="ExternalInput")
    wv_d = nc.dram_tensor("wv", (D, FH), F16, kind="ExternalInput")
    wo_d = nc.dram_tensor("wo", (D, D), F16, kind="ExternalInput")
    bq_d = nc.dram_tensor("bq", (NFG, 128, 1), F32, kind="ExternalInput")
    bk_d = nc.dram_tensor("bk", (NFG, 128, 1), F32, kind="ExternalInput")
    bv_d = nc.dram_tensor("bv", (NFG, 128, 1), F32, kind="ExternalInput")
    bo_d = nc.dram_tensor("bo", (8, 128, 1), F32, kind="ExternalInput")
    mask_d = nc.dram_tensor("mask4", (128, 2048), F16, kind="ExternalInput")
    id_d = nc.dram_tensor("ident", (128, 128), F16, kind="ExternalInput")
    out_d = nc.dram_tensor("out_T", (D, T // 2), F32, kind="ExternalOutput")

    with tile.TileContext(nc) as tc:
        with (
            tc.tile_pool(name="const", bufs=1) as cpool,
            tc.tile_pool(name="ofeat", bufs=4) as opool,
            tc.tile_pool(name="psA", bufs=1, space="PSUM") as psA,
            tc.tile_pool(name="psS", bufs=3, space="PSUM") as psS,
            tc.tile_pool(name="psO", bufs=1, space="PSUM") as psO,
            tc.tile_pool(name="dram", bufs=1, space="DRAM") as dram,
        ):
            mask4 = cpool.tile([128, 2048], F16, tag="mask")
            nc.sync.dma_start(mask4[:], mask_d[:])
            ident = cpool.tile([128, 128], F16, tag="ident")
            nc.sync.dma_start(ident[:], id_d[:])
            bqs, bks, bvs, bos = [], [], [], []
            for i in range(NFG):
                bqt = cpool.tile([128, 1], F32, tag=f"bq{i}")
                nc.sync.dma_start(bqt[:], bq_d[i])
                bqs.append(bqt)
                bkt = cpool.tile([128, 1], F32, tag=f"bk{i}")
                nc.sync.dma_start(bkt[:], bk_d[i])
                bks.append(bkt)
                bvt = cpool.tile([128, 1], F32, tag=f"bv{i}")
                nc.sync.dma_start(bvt[:], bv_d[i])
                bvs.append(bvt)
            for i in range(8):
                bot = cpool.tile([128, 1], F32, tag=f"bo{i}")
                nc.sync.dma_start(bot[:], bo_d[i])
                bos.append(bot)

            # O_feat: per-fg [128 feat, 2048 tok] fp16, feature-major
            o_feat = []
            probe_srcs = []
            if phase != "qkv":
                for fg in range(NFG):
                    of = opool.tile([128, T], F16, tag="ofeat")
                    o_feat.append(of)

            with (
                tc.tile_pool(name="wqkv", bufs=1) as wpool,
                tc.tile_pool(name="xt", bufs=8) as xpool,
                tc.tile_pool(name="qk", bufs=2) as qkpool,
                tc.tile_pool(name="vst", bufs=36) as vpool,
                tc.tile_pool(name="vstg", bufs=4) as vstgpool,
                tc.tile_pool(name="pp", bufs=18) as ppool,
                tc.tile_pool(name="otok", bufs=8) as otokpool,
                tc.tile_pool(name="misc", bufs=8) as mpool,
            ):
                # resident xT: 8 tiles [128 d, 2048 t]
                xts = []
                for dsub in range(NDS):
                    xt = xpool.tile([128, T], F16, tag="xt")
                    nc.sync.dma_start(xt[:], xT_d[128 * dsub : 128 * (dsub + 1), :])
                    xts.append(xt)
                # resident weights: per proj 8 tiles [128 d, 512 f]
                wts = {}
                for pname, wd in (("q", wq_d), ("k", wk_d), ("v", wv_d)):
                    for dsub in range(NDS):
                        wt = wpool.tile([128, FH], F16, tag=f"w{pname}{dsub}")
                        nc.sync.dma_start(
                            wt[:], wd[128 * dsub : 128 * (dsub + 1), :]
                        )
                        wts[(pname, dsub)] = wt

                for fg in range(NFG):
                    f0 = 128 * fg  # feature offset within this core's 512
                    # ---- Q/K projections (feature-major [128 f, 2048 t]) ----
                    # per-head Q/K with features duplicated across both
                    # partition halves (S then contracts over 128 partitions
                    # at full SBUF stream width; scores come out doubled).
                    qd = [qkpool.tile([128, T], F16, tag=f"qd{h}", name=f"qd{h}") for h in range(2)]
                    kd = [qkpool.tile([128, T], F16, tag=f"kd{h}", name=f"kd{h}") for h in range(2)]
                    for pname, dsts, bias in (("q", qd, bqs[fg]), ("k", kd, bks[fg])):
                        for tch in range(NTCH):
                            t0 = 512 * tch
                            ps = psA.tile([128, 512], F32, tag="proj")
                            for dsub in range(NDS):
                                nc.tensor.matmul(
                                    ps[:],
                                    wts[(pname, dsub)][:, f0 : f0 + 128],
                                    xts[dsub][:, t0 : t0 + 512],
                                    start=(dsub == 0),
                                    stop=(dsub == NDS - 1),
                                )
                            bap = bias[:]
                            nc.vector.tensor_scalar_add(
                                dsts[0][0:64, t0 : t0 + 512], ps[0:64, :], bap[0:64, :]
                            )
                            nc.vector.tensor_scalar_add(
                                dsts[1][64:128, t0 : t0 + 512], ps[64:128, :], bap[64:128, :]
                            )
                        # duplicate the written half into the other half
                        nc.sync.dma_start(dsts[0][64:128, :], dsts[0][0:64, :])
                        nc.sync.dma_start(dsts[1][0:64, :], dsts[1][64:128, :])
                    # ---- V projection -> token-major [128 t, 130] per t-tile ----
                    # cols: [head0 v(64) | 1.0 | head1 v(64) | 1.0]
                    vstore = []
                    for tt in range(NTT):
                        vt = vpool.tile([128, 130], F16, tag="vst")
                        nc.vector.memset(vt[:], 1.0)
                        vstore.append(vt)
                    for tch in range(NTCH):
                        t0 = 512 * tch
                        ps = psA.tile([128, 512], F32, tag="proj")
                        for dsub in range(NDS):
                            nc.tensor.matmul(
                                ps[:],
                                wts[("v", dsub)][:, f0 : f0 + 128],
                                xts[dsub][:, t0 : t0 + 512],
                                start=(dsub == 0),
                                stop=(dsub == NDS - 1),
                            )
                        vstg = vstgpool.tile([128, 512], F16, tag="vstg")
                        nc.vector.tensor_scalar_add(vstg[:], ps[:], bvs[fg][:])
                        for i in range(4):
                            tt = 4 * tch + i
                            pst = psA.tile([128, 128], F16, tag="proj")
                            nc.tensor.transpose(
                                pst[:], vstg[:, 128 * i : 128 * (i + 1)], ident[:]
                            )
                            nc.vector.tensor_copy(
                                vstore[tt][:].rearrange("p (h c) -> p h c", h=2)[
                                    :, :, 0:64
                                ],
                                pst[:].rearrange("p (h c) -> p h c", h=2),
                            )

                    # ---- attention for the 2 heads of this fg ----
                    if phase == "qkv":
                        probe_srcs.append((qf, kf, vstore[15]))
                        continue
                    # S_T in groups of 2 kblocks ([128, 1024] psum, double
                    # buffered) so the S-matmul stream runs ahead of exp.
                    # PV accumulates BOTH heads into one [128, 130] bank;
                    # normalize is one strided recip + one stride-0-broadcast
                    # multiply per query tile.
                    for j in range(NQC):
                        q0 = 512 * j
                        p_tiles = {}  # (hl, grp of 2 kblocks) -> [128,1024] f16
                        for hl in range(2):
                            for grp in range(2 * (j + 1)):
                                pss = psS.tile([128, 1024], F32, tag="s")
                                for ki in range(2):
                                    kb = 2 * grp + ki
                                    nc.tensor.matmul(
                                        pss[:, 512 * ki : 512 * (ki + 1)],
                                        kd[hl][:, 128 * kb : 128 * (kb + 1)],
                                        qd[hl][:, q0 : q0 + 512],
                                        start=True,
                                        stop=True,
                                    )
                                pt = ppool.tile([128, 1024], F16, tag="p")
                                nc.scalar.activation(
                                    pt[:],
                                    pss[:],
                                    mybir.ActivationFunctionType.Exp,
                                    scale=EXP_SCALE_DUP,
                                )
                                if grp >= 2 * j:  # diagonal groups (idle Pool engine)
                                    d = grp - 2 * j
                                    nc.gpsimd.tensor_mul(
                                        pt[:], pt[:], mask4[:, 1024 * d : 1024 * (d + 1)]
                                    )
                                p_tiles[(hl, grp)] = pt
                        for i in range(4):
                            qt = 4 * j + i
                            pso = psO.tile([128, 130], F32, tag="o")
                            nkb = 4 * j + i
                            for hl in range(2):
                                for kb in range(nkb + 1):
                                    grp, ki = kb // 2, kb % 2
                                    c0 = 512 * ki + 128 * i
                                    nc.tensor.matmul(
                                        pso[:, 65 * hl : 65 * hl + 65],
                                        p_tiles[(hl, grp)][:, c0 : c0 + 128],
                                        vstore[kb][:, 65 * hl : 65 * hl + 65],
                                        start=(kb == 0),
                                        stop=(kb == nkb),
                                    )
                            psv = pso[:].rearrange("p (h c) -> p h c", h=2)
                            rec = mpool.tile([128, 2], F32, tag="rec")
                            nc.vector.reciprocal(rec[:], psv[:, :, 64])
                            ot = otokpool.tile([128, 128], F16, tag="otok")
                            rec_b = bass.AP(
                                rec[:].tensor, rec[:].offset,
                                [rec[:].ap[0], [1, 2], [0, 64]],
                            )
                            nc.vector.tensor_tensor(
                                ot[:].rearrange("p (h c) -> p h c", h=2),
                                psv[:, :, 0:64],
                                rec_b,
                                mybir.AluOpType.mult,
                            )
                            pst = psA.tile([128, 128], F16, tag="proj")
                            nc.tensor.transpose(pst[:], ot[:], ident[:])
                            nc.vector.tensor_copy(
                                o_feat[fg][:, 128 * qt : 128 * (qt + 1)], pst[:]
                            )

            if phase in ("qkv", "attn"):
                with tc.tile_pool(name="probe", bufs=1) as prpool:
                    pr = prpool.tile([128, 512], F32, tag="pr")
                    if phase == "qkv":
                        q_, k_, v_ = probe_srcs[-1]
                        nc.vector.tensor_copy(pr[:, 0:128], q_[:, 0:128])
                        nc.vector.tensor_copy(pr[:, 128:256], k_[:, 0:128])
                        nc.vector.tensor_copy(pr[:, 256:321], v_[:, 0:65])
                    else:
                        nc.vector.tensor_copy(pr[:], o_feat[0][:, 0:512])
                    nc.sync.dma_start(out_d[0:128, 0:512], pr[:])
                nc.compile()
                return nc

            # ---- exchange: pairwise AllGather of full O_feat ----
            cc_in = dram.tile([FH, T], F16)
            cc_out = dram.tile([2, FH, T], F16)
            for fg in range(NFG):
                nc.sync.dma_start(cc_in[128 * fg : 128 * (fg + 1), :], o_feat[fg][:])
            if sim_mode:
                nc.sync.dma_start(cc_out[0], cc_in[:])
                nc.sync.dma_start(cc_out[1], cc_in[:])
                poff = 0
            else:
                nc.gpsimd.collective_compute(
                    "AllGather",
                    mybir.AluOpType.bypass,
                    replica_groups=[[0, 1], [2, 3], [4, 5], [6, 7]],
                    ins=[cc_in.opt()],
                    outs=[cc_out.opt()],
                )
                pid = nc.gpsimd.partition_id()
                poff = (pid % 2) * (T // 2)

            with (
                tc.tile_pool(name="att", bufs=8) as apool,
                tc.tile_pool(name="wo", bufs=8) as wopool,
                tc.tile_pool(name="outs", bufs=4) as outpool,
            ):
                att = []
                for s in range(2):
                    for fg in range(NFG):
                        at = apool.tile([128, T // 2], F16, tag="att")
                        if sim_mode:
                            nc.gpsimd.dma_start(
                                at[:],
                                cc_out[s][128 * fg : 128 * (fg + 1), 0 : T // 2],
                            )
                        else:
                            nc.gpsimd.dma_start(
                                at[:],
                                cc_out[s][128 * fg : 128 * (fg + 1), ds(poff, T // 2)],
                            )
                        att.append(at)
                wos = []
                for fs in range(8):
                    wt = wopool.tile([128, D], F16, tag="wo")
                    nc.sync.dma_start(wt[:], wo_d[128 * fs : 128 * (fs + 1), :])
                    wos.append(wt)
                for dt_ in range(8):
                    for tch in range(2):
                        t0 = 512 * tch
                        ps = psA.tile([128, 512], F32, tag="proj")
                        for fs in range(8):
                            nc.tensor.matmul(
                                ps[:],
                                wos[fs][:, 128 * dt_ : 128 * (dt_ + 1)],
                                att[fs][:, t0 : t0 + 512],
                                start=(fs == 0),
                                stop=(fs == 7),
                            )
                        ob = outpool.tile([128, 512], F32, tag="ob")
                        nc.vector.tensor_scalar_add(ob[:], ps[:], bos[dt_][:])
                        nc.sync.dma_start(
                            out_d[128 * dt_ : 128 * (dt_ + 1), t0 : t0 + 512], ob[:]
                        )

    nc.compile()
    return nc


def _prep_inputs(x, Wq, bq, Wk, bk, Wv, bv, Wo, bo):
    """Build the 8 per-core input maps."""
    x = np.asarray(x)
    mask4 = np.zeros((128, 2048), dtype=np.float16)
    r = np.arange(128)[:, None]
    for i in range(4):
        c = np.arange(512)[None, :]
        mask4[:, 512 * i : 512 * (i + 1)] = (c >= 128 * i + r).astype(np.float16)
    ident = np.eye(128, dtype=np.float16)
    wo16 = np.asarray(Wo).astype(np.float16)
    bo_r = np.asarray(bo).astype(np.float32).reshape(8, 128, 1)

    in_maps = []
    for c in range(N_CORES):
        b = c // 2
        hs = (c % 2) * FH
        in_maps.append(
            {
                "xT": np.ascontiguousarray(x[b].T).astype(np.float16),
                "wq": np.asarray(Wq)[:, hs : hs + FH].astype(np.float16),
                "wk": np.asarray(Wk)[:, hs : hs + FH].astype(np.float16),
                "wv": np.asarray(Wv)[:, hs : hs + FH].astype(np.float16),
                "wo": wo16,
                "bq": np.asarray(bq)[hs : hs + FH].astype(np.float32).reshape(4, 128, 1),
                "bk": np.asarray(bk)[hs : hs + FH].astype(np.float32).reshape(4, 128, 1),
                "bv": np.asarray(bv)[hs : hs + FH].astype(np.float32).reshape(4, 128, 1),
                "bo": bo_r,
                "mask4": mask4,
                "ident": ident,
            }
        )
    return in_maps


_NC_CACHE = None


def kernel(x, Wq, bq, Wk, bk, Wv, bv, Wo, bo):
    global _NC_CACHE
    if _NC_CACHE is None:
        _NC_CACHE = build_nc()
    nc = _NC_CACHE
    in_maps = _prep_inputs(x, Wq, bq, Wk, bk, Wv, bv, Wo, bo)
    res = bass_utils.run_bass_kernel_spmd(nc, in_maps, core_ids=list(range(N_CORES)))
    out = np.empty((B, T, D), dtype=np.float32)
    for c in range(N_CORES):
        b = c // 2
        half = c % 2
        out[b, half * (T // 2) : (half + 1) * (T // 2), :] = res.results[c]["out_T"].T
    return out



# revision 11
# speedup vs baseline: 1.5653x; 1.5653x over previous
"""Causal self-attention (B=4, T=2048, D=1024, H=16) on 8 TRN2 NeuronCores.

Sharding: data parallel over batch (4 batches x 2 core-pairs) and tensor
parallel over heads (8 heads per core). Each core:
  - projects its batch's tokens to Q/K (feature-major, per-head halves on
    partition halves) and V (token-major via x-stationary matmuls),
  - runs causal attention with per-head row-tiled S matmuls (K=64, both
    heads concurrent on disjoint PE row groups), causal masking via an
    additive -30000 upper-triangular matmul folded into the S psum
    accumulation (exp underflows to 0, no post-exp mask pass),
  - softmax without max-subtraction, denominators from a ones-column in V,
  - pairwise AllGather ships only the partner-needed token half; the out
    projection reads its own half straight from SBUF and accumulates
    own-half products before the collective lands.
Host reassembles the full (4, 2048, 1024) output.
"""

import numpy as np

import concourse.bass as bass
import concourse.mybir as mybir
import concourse.tile as tile
from concourse import bacc, bass_utils
from concourse.bass import ds

N_CORES = 8
B, T, D, H = 4, 2048, 1024, 16
HD = D // H  # 64
FH = 512  # features per core (8 heads)
NFG = 4  # feature groups of 128 (2 heads each) per core
NDS = 8  # 128-row contraction sub-tiles of D
NQC = 4  # 512-query chunks
NTT = 16  # 128-token tiles
TT2 = T // 2
F16 = mybir.dt.float16
BF16 = mybir.dt.bfloat16
F32 = mybir.dt.float32
EXP_SCALE = float(1.0 / np.sqrt(HD))
MASK_NEG = -30000.0


def build_nc(sim_mode=False):
    nc = bacc.Bacc("TRN2", target_bir_lowering=False, debug=False, num_devices=N_CORES)

    xT_d = nc.dram_tensor("xT", (D, T), F16, kind="ExternalInput")
    wq_d = nc.dram_tensor("wq", (D, FH), F16, kind="ExternalInput")
    wk_d = nc.dram_tensor("wk", (D, FH), F16, kind="ExternalInput")
    wv_d = nc.dram_tensor("wv", (D, FH), F16, kind="ExternalInput")
    woa_d = nc.dram_tensor("woa", (FH, D), F16, kind="ExternalInput")
    wob_d = nc.dram_tensor("wob", (FH, D), F16, kind="ExternalInput")
    bq_d = nc.dram_tensor("bq", (NFG, 128, 1), F32, kind="ExternalInput")
    bk_d = nc.dram_tensor("bk", (NFG, 128, 1), F32, kind="ExternalInput")
    bv_d = nc.dram_tensor("bvr", (1, FH), F16, kind="ExternalInput")
    bo_d = nc.dram_tensor("bo", (8, 128, 1), F32, kind="ExternalInput")
    idf_d = nc.dram_tensor("identf", (128, 128), F16, kind="ExternalInput")
    ut_d = nc.dram_tensor("utri", (128, 128), F16, kind="ExternalInput")
    out_d = nc.dram_tensor("out_T", (D, TT2), F32, kind="ExternalOutput")

    with tile.TileContext(nc) as tc:
        with (
            tc.tile_pool(name="const", bufs=1) as cpool,
            tc.tile_pool(name="ofeat", bufs=1) as opool,
            tc.tile_pool(name="psA", bufs=2, space="PSUM") as psA,
            tc.tile_pool(name="psS", bufs=2, space="PSUM") as psS,
            tc.tile_pool(name="psO", bufs=2, space="PSUM") as psO,
            tc.tile_pool(name="dram", bufs=1, space="DRAM") as dram,
        ):
            # small consts first (needed early)
            identf = cpool.tile([128, 128], F16, tag="identf")
            nc.sync.dma_start(identf[:], idf_d[:])
            utri = cpool.tile([128, 128], F16, tag="utri")
            nc.sync.dma_start(utri[:], ut_d[:])
            ones1 = cpool.tile([1, 128], F16, tag="ones1")
            nc.vector.memset(ones1[:], 1.0)
            bqs, bks, bos = [], [], []
            for i in range(NFG):
                bqt = cpool.tile([128, 1], F32, tag=f"bq{i}")
                nc.sync.dma_start(bqt[:], bq_d[i])
                bqs.append(bqt)
                bkt = cpool.tile([128, 1], F32, tag=f"bk{i}")
                nc.sync.dma_start(bkt[:], bk_d[i])
                bks.append(bkt)
            bvr = cpool.tile([1, FH], F16, tag="bvr")
            nc.sync.dma_start(bvr[:], bv_d[:])
            for i in range(8):
                bot = cpool.tile([128, 1], F32, tag=f"bo{i}")
                nc.sync.dma_start(bot[:], bo_d[i])
                bos.append(bot)

            # O_feat: per-fg [128 feat, 2048 tok] fp16, feature-major
            o_feat = []
            for fg in range(NFG):
                of = opool.tile([128, T], F16, tag=f"ofeat{fg}", name=f"ofeat{fg}")
                o_feat.append(of)

            with (
                tc.tile_pool(name="wqkv", bufs=1) as wpool,
                tc.tile_pool(name="xt", bufs=1) as xpool,
                tc.tile_pool(name="qk", bufs=2) as qkpool,
                tc.tile_pool(name="vst", bufs=1) as vpool,
                tc.tile_pool(name="pp", bufs=18) as ppool,
                tc.tile_pool(name="misc", bufs=8) as mpool,
                tc.tile_pool(name="wo", bufs=1) as wopool,
                tc.tile_pool(name="att", bufs=1) as apool,
                tc.tile_pool(name="outs", bufs=4) as outpool,
            ):
                # resident xT (interleaved with wv so V-proj can start early)
                xts, wts = [], {}
                for dsub in range(NDS):
                    xt = xpool.tile([128, T], F16, tag=f"xt{dsub}")
                    nc.sync.dma_start(xt[:], xT_d[128 * dsub : 128 * (dsub + 1), :])
                    xts.append(xt)
                    wt = wpool.tile([128, FH], F16, tag=f"wv{dsub}")
                    nc.sync.dma_start(wt[:], wv_d[128 * dsub : 128 * (dsub + 1), :])
                    wts[("v", dsub)] = wt
                for pname, wd in (("q", wq_d), ("k", wk_d)):
                    for dsub in range(NDS):
                        wt = wpool.tile([128, FH], F16, tag=f"w{pname}{dsub}")
                        nc.sync.dma_start(wt[:], wd[128 * dsub : 128 * (dsub + 1), :])
                        wts[(pname, dsub)] = wt
                # resident Wo (own rows + partner rows), prefetched up front
                woa, wob = [], []
                for fs in range(4):
                    wt = wopool.tile([128, D], F16, tag=f"woa{fs}")
                    nc.sync.dma_start(wt[:], woa_d[128 * fs : 128 * (fs + 1), :])
                    woa.append(wt)
                for fs in range(4):
                    wt = wopool.tile([128, D], F16, tag=f"wob{fs}")
                    nc.sync.dma_start(wt[:], wob_d[128 * fs : 128 * (fs + 1), :])
                    wob.append(wt)

                # V store: per t-tile [128 tok, 520]: 8x(64 v | 1.0)
                vstore = []
                for tt in range(NTT):
                    vt = vpool.tile([128, 8 * 65], BF16, tag=f"vst{tt}")
                    vstore.append(vt)

                def v_chain(tt):
                    ps = psA.tile([128, FH], F32, tag="proj")
                    for dsub in range(NDS):
                        nc.tensor.matmul(
                            ps[:],
                            xts[dsub][:, 128 * tt : 128 * (tt + 1)],
                            wts[("v", dsub)][:],
                            start=(dsub == 0),
                            stop=False,
                        )
                    # bias row broadcast along tokens via K=1 ones matmul
                    nc.tensor.matmul(ps[:], ones1[:], bvr[:], start=False, stop=True)
                    nc.gpsimd.memset(vstore[tt][:], 1.0)
                    nc.vector.tensor_copy(
                        vstore[tt][:].rearrange("p (g c) -> p g c", g=8)[:, :, 0:64],
                        ps[:].rearrange("p (g c) -> p g c", g=8),
                    )

                def qk_chain(pname, fg, tch, dst, bias):
                    f0 = 128 * fg
                    t0 = 512 * tch
                    ps = psA.tile([128, 512], F32, tag="proj")
                    for dsub in range(NDS):
                        nc.tensor.matmul(
                            ps[:],
                            wts[(pname, dsub)][:, f0 : f0 + 128],
                            xts[dsub][:, t0 : t0 + 512],
                            start=(dsub == 0),
                            stop=(dsub == NDS - 1),
                        )
                    nc.vector.tensor_scalar_add(dst[:, t0 : t0 + 512], ps[:], bias[:])

                # filler worklist: emitters for PE work to slot into the
                # ACT-bound attention phase
                filler = []

                def fill(k=1):
                    for _ in range(k):
                        if filler:
                            filler.pop(0)()

                # upfront: V t-tiles 0..7, fg0 Q/K
                qkd = {}  # (fg, 'q'/'k') -> tile
                qkd[(0, "q")] = qkpool.tile([128, T], F16, tag="qd", name="qd0")
                qkd[(0, "k")] = qkpool.tile([128, T], F16, tag="kd", name="kd0")
                for tt in range(8):
                    v_chain(tt)
                for pname in ("q", "k"):
                    for tch in range(4):
                        qk_chain(pname, 0, tch, qkd[(0, pname)],
                                 (bqs if pname == "q" else bks)[0])
                # defer V t-tiles 8..15 into fg0's attention
                for tt in range(8, NTT):
                    filler.append(lambda tt=tt: v_chain(tt))

                cc_bufs = []  # (cc_out_flat, sem info) per half
                pid = None if sim_mode else nc.gpsimd.partition_id()
                # token offset of this core's half (0 or 1024)
                if sim_mode:
                    poff, opp, roff = 0, TT2, 256
                else:
                    poff = (pid % 2) * TT2
                    opp = ((pid + 1) % 2) * TT2
                    roff = ((pid + 1) % 2) * 256

                att_own = []

                for fg in range(NFG):
                    # allocate next fg's q/k tiles and queue its proj chains
                    if fg + 1 < NFG:
                        qkd[(fg + 1, "q")] = qkpool.tile([128, T], F16, tag="qd", name=f"qd{fg+1}")
                        qkd[(fg + 1, "k")] = qkpool.tile([128, T], F16, tag="kd", name=f"kd{fg+1}")
                        for pname in ("q", "k"):
                            for tch in range(4):
                                filler.append(
                                    lambda p=pname, f=fg + 1, t=tch: qk_chain(
                                        p, f, t, qkd[(f, p)],
                                        (bqs if p == "q" else bks)[f])
                                )
                    qd, kd = qkd[(fg, "q")], qkd[(fg, "k")]

                    for j in range(NQC):
                        q0 = 512 * j
                        ngrp = 2 * (j + 1)
                        p_tiles = {}
                        for grp in range(ngrp):
                            pss = {}
                            for hl in range(2):
                                pss[hl] = psS.tile([128, 1024], F32, tag="s", name=f"s{hl}")
                            # interleave heads for row-group concurrency
                            for ki in range(2):
                                kb = 2 * grp + ki
                                diag = kb >= 4 * j  # triangular tile on diagonal
                                if diag:
                                    c0 = 512 * ki + 128 * (kb - 4 * j)
                                    for hl in range(2):
                                        nc.tensor.matmul(
                                            pss[hl][:, c0 : c0 + 128],
                                            identf[:],
                                            utri[:],
                                            start=True,
                                            stop=False,
                                        )
                                for hl in range(2):
                                    h0 = 64 * hl
                                    nc.tensor.matmul(
                                        pss[hl][:, 512 * ki : 512 * (ki + 1)],
                                        kd[h0 : h0 + 64, 128 * kb : 128 * (kb + 1)],
                                        qd[h0 : h0 + 64, q0 : q0 + 512],
                                        start=not diag,
                                        stop=True,
                                    )
                            for hl in range(2):
                                pt = ppool.tile([128, 1024], BF16, tag="p")
                                nc.scalar.activation(
                                    pt[:],
                                    pss[hl][:],
                                    mybir.ActivationFunctionType.Exp,
                                    scale=EXP_SCALE,
                                )
                                p_tiles[(hl, grp)] = pt
                            if grp % 2 == 1:
                                fill(1)
                        for i in range(4):
                            qt = 4 * j + i
                            pso = psO.tile([128, 130], F32, tag="o")
                            nkb = 4 * j + i
                            for hl in range(2):
                                for kb in range(nkb + 1):
                                    grp, ki = kb // 2, kb % 2
                                    c0 = 512 * ki + 128 * i
                                    nc.tensor.matmul(
                                        pso[:, 65 * hl : 65 * hl + 65],
                                        p_tiles[(hl, grp)][:, c0 : c0 + 128],
                                        vstore[kb][:, 130 * fg + 65 * hl : 130 * fg + 65 * hl + 65],
                                        start=(kb == 0),
                                        stop=(kb == nkb),
                                    )
                            psv = pso[:].rearrange("p (h c) -> p h c", h=2)
                            rec = mpool.tile([128, 2], F32, tag="rec")
                            nc.vector.reciprocal(rec[:], psv[:, :, 64])
                            ot = mpool.tile([128, 128], F16, tag="otok")
                            rec_b = bass.AP(
                                rec[:].tensor, rec[:].offset,
                                [rec[:].ap[0], [1, 2], [0, 64]],
                            )
                            nc.vector.tensor_tensor(
                                ot[:].rearrange("p (h c) -> p h c", h=2),
                                psv[:, :, 0:64],
                                rec_b,
                                mybir.AluOpType.mult,
                            )
                            pst = psO.tile([128, 128], F16, tag="o")
                            nc.tensor.transpose(pst[:], ot[:], identf[:])
                            nc.vector.tensor_copy(
                                o_feat[fg][:, 128 * qt : 128 * (qt + 1)], pst[:]
                            )
                            fill(1)

                    # own-half staging for the out projection
                    at = apool.tile([128, TT2], F16, tag=f"attown{fg}")
                    if sim_mode:
                        nc.sync.dma_start(at[:], o_feat[fg][:, 0:TT2])
                    else:
                        nc.gpsimd.dma_start(at[:], o_feat[fg][:, ds(poff, TT2)])
                    att_own.append(at)

                    # pairwise exchange of the partner-needed halves,
                    # split into two collectives (fg 0-1, fg 2-3)
                    if fg in (1, 3):
                        cc_in = dram.tile([256, TT2], F16)
                        cc_out = dram.tile([2, 256, TT2], F16)
                        for k, fgi in enumerate((fg - 1, fg)):
                            if sim_mode:
                                nc.sync.dma_start(
                                    cc_in[128 * k : 128 * (k + 1), :],
                                    o_feat[fgi][:, TT2 : T],
                                )
                            else:
                                nc.gpsimd.dma_start(
                                    cc_in[128 * k : 128 * (k + 1), :],
                                    o_feat[fgi][:, ds(opp, TT2)],
                                )
                        if sim_mode:
                            nc.sync.dma_start(cc_out[0], cc_in[:])
                            nc.sync.dma_start(cc_out[1], cc_in[:])
                        else:
                            nc.gpsimd.collective_compute(
                                "AllGather",
                                mybir.AluOpType.bypass,
                                replica_groups=[[0, 1], [2, 3], [4, 5], [6, 7]],
                                ins=[cc_in.opt()],
                                outs=[cc_out.opt()],
                            )
                        cc_bufs.append(cc_out)

                # drain any leftover filler work
                fill(len(filler))

                # partner-half attention tiles from the exchanged buffers
                att_p = []
                for half in range(2):
                    cc_flat = cc_bufs[half][:].rearrange("s p t -> (s p) t")
                    for k in range(2):
                        at = apool.tile([128, TT2], F16, tag=f"attp{2*half+k}")
                        if sim_mode:
                            nc.gpsimd.dma_start(
                                at[:], cc_flat[256 + 128 * k : 256 + 128 * (k + 1), :]
                            )
                        else:
                            nc.gpsimd.dma_start(
                                at[:], cc_flat[ds(roff + 128 * k, 128), :]
                            )
                        att_p.append(at)

                # out projection: own-half products first, partner after
                for dt_ in range(8):
                    for tch in range(2):
                        t0 = 512 * tch
                        ps = psA.tile([128, 512], F32, tag="proj")
                        for fs in range(4):
                            nc.tensor.matmul(
                                ps[:],
                                woa[fs][:, 128 * dt_ : 128 * (dt_ + 1)],
                                att_own[fs][:, t0 : t0 + 512],
                                start=(fs == 0),
                                stop=False,
                            )
                        for fs in range(4):
                            nc.tensor.matmul(
                                ps[:],
                                wob[fs][:, 128 * dt_ : 128 * (dt_ + 1)],
                                att_p[fs][:, t0 : t0 + 512],
                                start=False,
                                stop=(fs == 3),
                            )
                        ob = outpool.tile([128, 512], F32, tag="ob")
                        nc.vector.tensor_scalar_add(ob[:], ps[:], bos[dt_][:])
                        nc.sync.dma_start(
                            out_d[128 * dt_ : 128 * (dt_ + 1), t0 : t0 + 512], ob[:]
                        )

    nc.compile()
    return nc


def _prep_inputs(x, Wq, bq, Wk, bk, Wv, bv, Wo, bo):
    """Build the 8 per-core input maps."""
    x = np.asarray(x)
    ident = np.eye(128, dtype=np.float32)
    r = np.arange(128)
    utri = np.where(r[None, :] < r[:, None], np.float32(MASK_NEG), np.float32(0.0))
    bo_r = np.asarray(bo).astype(np.float32).reshape(8, 128, 1)
    Wo = np.asarray(Wo).astype(np.float16)

    in_maps = []
    for c in range(N_CORES):
        b = c // 2
        hs = (c % 2) * FH
        ps = FH - hs  # partner's feature offset
        in_maps.append(
            {
                "xT": np.ascontiguousarray(x[b].T).astype(np.float16),
                "wq": np.asarray(Wq)[:, hs : hs + FH].astype(np.float16),
                "wk": np.asarray(Wk)[:, hs : hs + FH].astype(np.float16),
                "wv": np.asarray(Wv)[:, hs : hs + FH].astype(np.float16),
                "woa": np.ascontiguousarray(Wo[hs : hs + FH, :]),
                "wob": np.ascontiguousarray(Wo[ps : ps + FH, :]),
                "bq": np.asarray(bq)[hs : hs + FH].astype(np.float32).reshape(4, 128, 1),
                "bk": np.asarray(bk)[hs : hs + FH].astype(np.float32).reshape(4, 128, 1),
                "bvr": np.asarray(bv)[hs : hs + FH].astype(np.float16).reshape(1, FH),
                "bo": bo_r,
                "identf": ident.astype(np.float16),
                "utri": utri.astype(np.float16),
            }
        )
    return in_maps


_NC_CACHE = None


def kernel(x, Wq, bq, Wk, bk, Wv, bv, Wo, bo):
    global _NC_CACHE
    if _NC_CACHE is None:
        _NC_CACHE = build_nc()
    nc = _NC_CACHE
    in_maps = _prep_inputs(x, Wq, bq, Wk, bk, Wv, bv, Wo, bo)
    res = bass_utils.run_bass_kernel_spmd(nc, in_maps, core_ids=list(range(N_CORES)))
    out = np.empty((B, T, D), dtype=np.float32)
    for c in range(N_CORES):
        b = c // 2
        half = c % 2
        out[b, half * TT2 : (half + 1) * TT2, :] = res.results[c]["out_T"].T
    return out


# revision 17
# speedup vs baseline: 3.5268x; 2.2531x over previous
"""Causal self-attention (B=4, T=2048, D=1024, H=16) on 8 TRN2 NeuronCores.

Sharding: data parallel over batch (4 batches x 2 core-pairs) and tensor
parallel over heads (8 heads per core). Each core:
  - projects its batch's tokens to Q/K (feature-major, per-head halves on
    partition halves) and V (token-major via x-stationary matmuls),
  - runs causal attention with per-head row-tiled S matmuls (K=64, both
    heads concurrent on disjoint PE row groups), causal masking via an
    additive -30000 upper-triangular matmul folded into the S psum
    accumulation (exp underflows to 0, no post-exp mask pass),
  - softmax without max-subtraction, denominators from a ones-column in V,
  - pairwise AllGather ships only the partner-needed token half; the out
    projection reads its own half straight from SBUF and accumulates
    own-half products before the collective lands.
Host reassembles the full (4, 2048, 1024) output.
"""

import numpy as np

import concourse.bass as bass
import concourse.mybir as mybir
import concourse.tile as tile
from concourse import bacc, bass_utils
from concourse.bass import ds

N_CORES = 8
B, T, D, H = 4, 2048, 1024, 16
HD = D // H  # 64
FH = 512  # features per core (8 heads)
NFG = 4  # feature groups of 128 (2 heads each) per core
NDS = 8  # 128-row contraction sub-tiles of D
NQC = 4  # 512-query chunks
NTT = 16  # 128-token tiles
TT2 = T // 2
F16 = mybir.dt.float16
BF16 = mybir.dt.bfloat16
F32 = mybir.dt.float32
EXP_SCALE = float(1.0 / np.sqrt(HD))
MASK_NEG = -30000.0


def build_nc(sim_mode=False):
    nc = bacc.Bacc("TRN2", target_bir_lowering=False, debug=False, num_devices=N_CORES)

    xT_d = nc.dram_tensor("xT", (D, T), F16, kind="ExternalInput")
    wq_d = nc.dram_tensor("wq", (D, FH), F16, kind="ExternalInput")
    wk_d = nc.dram_tensor("wk", (D, FH), F16, kind="ExternalInput")
    wv_d = nc.dram_tensor("wv", (D, FH), F16, kind="ExternalInput")
    woa_d = nc.dram_tensor("woa", (FH, D), F16, kind="ExternalInput")
    wob_d = nc.dram_tensor("wob", (FH, D), F16, kind="ExternalInput")
    bq_d = nc.dram_tensor("bq", (NFG, 128, 1), F32, kind="ExternalInput")
    bk_d = nc.dram_tensor("bk", (NFG, 128, 1), F32, kind="ExternalInput")
    bv_d = nc.dram_tensor("bvr", (1, FH), F16, kind="ExternalInput")
    bo_d = nc.dram_tensor("bo", (8, 128, 1), F32, kind="ExternalInput")
    idf_d = nc.dram_tensor("identf", (128, 128), F16, kind="ExternalInput")
    ut_d = nc.dram_tensor("utri", (128, 128), F16, kind="ExternalInput")
    out_d = nc.dram_tensor("out_T", (D, TT2), F32, kind="ExternalOutput")

    with tile.TileContext(nc) as tc:
        with (
            tc.tile_pool(name="const", bufs=1) as cpool,
            tc.tile_pool(name="ofeat", bufs=1) as opool,
            tc.tile_pool(name="psA", bufs=2, space="PSUM") as psA,
            tc.tile_pool(name="psS", bufs=2, space="PSUM") as psS,
            tc.tile_pool(name="psO", bufs=2, space="PSUM") as psO,
            tc.tile_pool(name="dram", bufs=1, space="DRAM") as dram,
        ):
            # small consts first (needed early)
            identf = cpool.tile([128, 128], F16, tag="identf")
            nc.sync.dma_start(identf[:], idf_d[:])
            utri = cpool.tile([128, 128], F16, tag="utri")
            nc.sync.dma_start(utri[:], ut_d[:])
            ones1 = cpool.tile([1, 128], F16, tag="ones1")
            nc.vector.memset(ones1[:], 1.0)
            bqs, bks, bos = [], [], []
            for i in range(NFG):
                bqt = cpool.tile([128, 1], F32, tag=f"bq{i}")
                nc.sync.dma_start(bqt[:], bq_d[i])
                bqs.append(bqt)
                bkt = cpool.tile([128, 1], F32, tag=f"bk{i}")
                nc.sync.dma_start(bkt[:], bk_d[i])
                bks.append(bkt)
            bvr = cpool.tile([1, FH], F16, tag="bvr")
            nc.sync.dma_start(bvr[:], bv_d[:])
            for i in range(8):
                bot = cpool.tile([128, 1], F32, tag=f"bo{i}")
                nc.sync.dma_start(bot[:], bo_d[i])
                bos.append(bot)

            # O_feat: per-fg [128 feat, 2048 tok] fp16, feature-major
            o_feat = []
            for fg in range(NFG):
                of = opool.tile([128, T], F16, tag=f"ofeat{fg}", name=f"ofeat{fg}")
                o_feat.append(of)

            with (
                tc.tile_pool(name="wqkv", bufs=1) as wpool,
                tc.tile_pool(name="xt", bufs=1) as xpool,
                tc.tile_pool(name="qk", bufs=2) as qkpool,
                tc.tile_pool(name="vst", bufs=1) as vpool,
                tc.tile_pool(name="pp", bufs=18) as ppool,
                tc.tile_pool(name="misc", bufs=8) as mpool,
                tc.tile_pool(name="wo", bufs=1) as wopool,
                tc.tile_pool(name="att", bufs=1) as apool,
                tc.tile_pool(name="outs", bufs=4) as outpool,
            ):
                # resident xT (column-chunked, interleaved with wv so the
                # first V/QK chains start after ~1/4 of the x traffic)
                xts, wts = [], {}
                for dsub in range(NDS):
                    xt = xpool.tile([128, T], F16, tag=f"xt{dsub}")
                    nc.sync.dma_start(
                        xt[:, 0:512], xT_d[128 * dsub : 128 * (dsub + 1), 0:512]
                    )
                    wt = wpool.tile([128, FH], F16, tag=f"wv{dsub}")
                    nc.sync.dma_start(wt[:], wv_d[128 * dsub : 128 * (dsub + 1), :])
                    xts.append(xt)
                    wts[("v", dsub)] = wt
                for chunk in range(1, 4):
                    t0c = 512 * chunk
                    for dsub in range(NDS):
                        nc.sync.dma_start(
                            xts[dsub][:, t0c : t0c + 512],
                            xT_d[128 * dsub : 128 * (dsub + 1), t0c : t0c + 512],
                        )
                for pname, wd in (("q", wq_d), ("k", wk_d)):
                    for dsub in range(NDS):
                        wt = wpool.tile([128, FH], F16, tag=f"w{pname}{dsub}")
                        nc.sync.dma_start(wt[:], wd[128 * dsub : 128 * (dsub + 1), :])
                        wts[(pname, dsub)] = wt
                # resident Wo (own rows + partner rows), prefetched up front
                woa, wob = [], []
                for fs in range(4):
                    wt = wopool.tile([128, D], F16, tag=f"woa{fs}")
                    nc.sync.dma_start(wt[:], woa_d[128 * fs : 128 * (fs + 1), :])
                    woa.append(wt)
                for fs in range(4):
                    wt = wopool.tile([128, D], F16, tag=f"wob{fs}")
                    nc.sync.dma_start(wt[:], wob_d[128 * fs : 128 * (fs + 1), :])
                    wob.append(wt)

                # V store: per t-tile [128 tok, 520]: 8x(64 v | 1.0)
                vstore = []
                for tt in range(NTT):
                    vt = vpool.tile([128, 8 * 65], F16, tag=f"vst{tt}")
                    vstore.append(vt)

                def v_chain(tt):
                    ps = psA.tile([128, FH], F32, tag="proj")
                    for dsub in range(NDS):
                        nc.tensor.matmul(
                            ps[:],
                            xts[dsub][:, 128 * tt : 128 * (tt + 1)],
                            wts[("v", dsub)][:],
                            start=(dsub == 0),
                            stop=False,
                        )
                    # bias row broadcast along tokens via K=1 ones matmul
                    nc.tensor.matmul(ps[:], ones1[:], bvr[:], start=False, stop=True)
                    nc.gpsimd.memset(vstore[tt][:], 1.0)
                    nc.vector.tensor_copy(
                        vstore[tt][:].rearrange("p (g c) -> p g c", g=8)[:, :, 0:64],
                        ps[:].rearrange("p (g c) -> p g c", g=8),
                    )

                def qk_chain(pname, fg, tch, dst, bias):
                    f0 = 128 * fg
                    t0 = 512 * tch
                    ps = psA.tile([128, 512], F32, tag="proj")
                    for dsub in range(NDS):
                        nc.tensor.matmul(
                            ps[:],
                            wts[(pname, dsub)][:, f0 : f0 + 128],
                            xts[dsub][:, t0 : t0 + 512],
                            start=(dsub == 0),
                            stop=(dsub == NDS - 1),
                        )
                    nc.vector.tensor_scalar_add(dst[:, t0 : t0 + 512], ps[:], bias[:])

                # filler worklist: emitters for PE work to slot into the
                # ACT-bound attention phase
                filler = []

                def fill(k=1):
                    for _ in range(k):
                        if filler:
                            filler.pop(0)()

                # upfront: V t-tiles 0..7, fg0 Q/K
                qkd = {}  # (fg, 'q'/'k') -> tile
                qkd[(0, "q")] = qkpool.tile([128, T], F16, tag="qd", name="qd0")
                qkd[(0, "k")] = qkpool.tile([128, T], F16, tag="kd", name="kd0")
                for tt in range(8):
                    v_chain(tt)
                for pname in ("q", "k"):
                    for tch in range(4):
                        qk_chain(pname, 0, tch, qkd[(0, pname)],
                                 (bqs if pname == "q" else bks)[0])
                # defer V t-tiles 8..15 into fg0's attention
                for tt in range(8, NTT):
                    filler.append(lambda tt=tt: v_chain(tt))

                cc_bufs = []  # (cc_out_flat, sem info) per half
                pid = None if sim_mode else nc.gpsimd.partition_id()
                # token offset of this core's half (0 or 1024)
                if sim_mode:
                    poff, opp, roff = 0, TT2, 256
                else:
                    poff = (pid % 2) * TT2
                    opp = ((pid + 1) % 2) * TT2
                    roff = ((pid + 1) % 2) * 256

                att_own = []

                for fg in range(NFG):
                    # allocate next fg's q/k tiles and queue its proj chains
                    if fg + 1 < NFG:
                        qkd[(fg + 1, "q")] = qkpool.tile([128, T], F16, tag="qd", name=f"qd{fg+1}")
                        qkd[(fg + 1, "k")] = qkpool.tile([128, T], F16, tag="kd", name=f"kd{fg+1}")
                        for pname in ("q", "k"):
                            for tch in range(4):
                                filler.append(
                                    lambda p=pname, f=fg + 1, t=tch: qk_chain(
                                        p, f, t, qkd[(f, p)],
                                        (bqs if p == "q" else bks)[f])
                                )
                    qd, kd = qkd[(fg, "q")], qkd[(fg, "k")]

                    def s_group(j, grp, qd, kd):
                        """Emit one [128k x 1024q] S psum group per head."""
                        q0 = 512 * j
                        pss = {}
                        for hl in range(2):
                            pss[hl] = psS.tile([128, 1024], F32, tag="s", name=f"s{hl}")
                        for ki in range(2):
                            kb = 2 * grp + ki
                            diag = kb >= 4 * j  # triangular tile on diagonal
                            if diag:
                                c0 = 512 * ki + 128 * (kb - 4 * j)
                                for hl in range(2):
                                    nc.tensor.matmul(
                                        pss[hl][:, c0 : c0 + 128],
                                        identf[:],
                                        utri[:],
                                        start=True,
                                        stop=False,
                                    )
                            for hl in range(2):
                                h0 = 64 * hl
                                nc.tensor.matmul(
                                    pss[hl][:, 512 * ki : 512 * (ki + 1)],
                                    kd[h0 : h0 + 64, 128 * kb : 128 * (kb + 1)],
                                    qd[h0 : h0 + 64, q0 : q0 + 512],
                                    start=not diag,
                                    stop=True,
                                )
                        return pss

                    for j in range(NQC):
                        ngrp = 2 * (j + 1)
                        p_tiles = {}
                        for grp in range(ngrp):
                            pss = s_group(j, grp, qd, kd)
                            for hl in range(2):
                                pt = ppool.tile([128, 1024], F16, tag="p")
                                nc.scalar.activation(
                                    pt[:],
                                    pss[hl][:],
                                    mybir.ActivationFunctionType.Exp,
                                    scale=EXP_SCALE,
                                )
                                p_tiles[(hl, grp)] = pt
                            if grp % 2 == 1:
                                fill(1)
                        for i in range(4):
                            qt = 4 * j + i
                            pso = psO.tile([128, 130], F32, tag="o")
                            nkb = 4 * j + i
                            for hl in range(2):
                                for kb in range(nkb + 1):
                                    grp, ki = kb // 2, kb % 2
                                    c0 = 512 * ki + 128 * i
                                    nc.tensor.matmul(
                                        pso[:, 65 * hl : 65 * hl + 65],
                                        p_tiles[(hl, grp)][:, c0 : c0 + 128],
                                        vstore[kb][:, 130 * fg + 65 * hl : 130 * fg + 65 * hl + 65],
                                        start=(kb == 0),
                                        stop=(kb == nkb),
                                    )
                            psv = pso[:].rearrange("p (h c) -> p h c", h=2)
                            rec = mpool.tile([128, 2], F32, tag="rec")
                            nc.vector.reciprocal(rec[:], psv[:, :, 64])
                            ot = mpool.tile([128, 128], F16, tag="otok")
                            rec_b = bass.AP(
                                rec[:].tensor, rec[:].offset,
                                [rec[:].ap[0], [1, 2], [0, 64]],
                            )
                            nc.vector.tensor_tensor(
                                ot[:].rearrange("p (h c) -> p h c", h=2),
                                psv[:, :, 0:64],
                                rec_b,
                                mybir.AluOpType.mult,
                            )
                            pst = psO.tile([128, 128], F16, tag="o")
                            nc.tensor.transpose(pst[:], ot[:], identf[:])
                            nc.vector.tensor_copy(
                                o_feat[fg][:, 128 * qt : 128 * (qt + 1)], pst[:]
                            )
                            fill(1)

                    # own-half staging for the out projection
                    at = apool.tile([128, TT2], F16, tag=f"attown{fg}")
                    if sim_mode:
                        nc.sync.dma_start(at[:], o_feat[fg][:, 0:TT2])
                    else:
                        nc.gpsimd.dma_start(at[:], o_feat[fg][:, ds(poff, TT2)])
                    att_own.append(at)

                    # pairwise exchange of the partner-needed halves, split
                    # into three collectives (fg 0-1 mid-kernel, fg2, fg3)
                    # so only the last small one can expose latency
                    if fg in (1, 2, 3):
                        fgis = (0, 1) if fg == 1 else (fg,)
                        nr = 128 * len(fgis)
                        cc_in = dram.tile([nr, TT2], F16)
                        cc_out = dram.tile([2, nr, TT2], F16)
                        for k, fgi in enumerate(fgis):
                            if sim_mode:
                                nc.sync.dma_start(
                                    cc_in[128 * k : 128 * (k + 1), :],
                                    o_feat[fgi][:, TT2 : T],
                                )
                            else:
                                nc.gpsimd.dma_start(
                                    cc_in[128 * k : 128 * (k + 1), :],
                                    o_feat[fgi][:, ds(opp, TT2)],
                                )
                        if sim_mode:
                            nc.sync.dma_start(cc_out[0], cc_in[:])
                            nc.sync.dma_start(cc_out[1], cc_in[:])
                        else:
                            nc.gpsimd.collective_compute(
                                "AllGather",
                                mybir.AluOpType.bypass,
                                replica_groups=[[0, 1], [2, 3], [4, 5], [6, 7]],
                                ins=[cc_in.opt()],
                                outs=[cc_out.opt()],
                            )
                        cc_bufs.append((cc_out, nr))

                # drain any leftover filler work
                fill(len(filler))

                # partner-half attention tiles from the exchanged buffers
                att_p = []
                for bi, (cc_out, nr) in enumerate(cc_bufs):
                    cc_flat = cc_out[:].rearrange("s p t -> (s p) t")
                    pbase = nr if sim_mode else ((pid + 1) % 2) * nr
                    for k in range(nr // 128):
                        at = apool.tile([128, TT2], F16, tag=f"attp{len(att_p)}")
                        if sim_mode:
                            nc.gpsimd.dma_start(
                                at[:], cc_flat[nr + 128 * k : nr + 128 * (k + 1), :]
                            )
                        else:
                            nc.gpsimd.dma_start(
                                at[:], cc_flat[ds(pbase + 128 * k, 128), :]
                            )
                        att_p.append(at)

                # out projection: own-half products first, partner after
                for dt_ in range(8):
                    for tch in range(2):
                        t0 = 512 * tch
                        ps = psA.tile([128, 512], F32, tag="proj")
                        for fs in range(4):
                            nc.tensor.matmul(
                                ps[:],
                                woa[fs][:, 128 * dt_ : 128 * (dt_ + 1)],
                                att_own[fs][:, t0 : t0 + 512],
                                start=(fs == 0),
                                stop=False,
                            )
                        for fs in range(4):
                            nc.tensor.matmul(
                                ps[:],
                                wob[fs][:, 128 * dt_ : 128 * (dt_ + 1)],
                                att_p[fs][:, t0 : t0 + 512],
                                start=False,
                                stop=(fs == 3),
                            )
                        ob = outpool.tile([128, 512], F32, tag="ob")
                        nc.vector.tensor_scalar_add(ob[:], ps[:], bos[dt_][:])
                        nc.sync.dma_start(
                            out_d[128 * dt_ : 128 * (dt_ + 1), t0 : t0 + 512], ob[:]
                        )

    nc.compile()
    return nc


def _prep_inputs(x, Wq, bq, Wk, bk, Wv, bv, Wo, bo):
    """Build the 8 per-core input maps."""
    x = np.asarray(x)
    ident = np.eye(128, dtype=np.float32)
    r = np.arange(128)
    utri = np.where(r[None, :] < r[:, None], np.float32(MASK_NEG), np.float32(0.0))
    bo_r = np.asarray(bo).astype(np.float32).reshape(8, 128, 1)
    Wo = np.asarray(Wo).astype(np.float16)

    in_maps = []
    for c in range(N_CORES):
        b = c // 2
        hs = (c % 2) * FH
        ps = FH - hs  # partner's feature offset
        in_maps.append(
            {
                "xT": np.ascontiguousarray(x[b].T).astype(np.float16),
                "wq": np.asarray(Wq)[:, hs : hs + FH].astype(np.float16),
                "wk": np.asarray(Wk)[:, hs : hs + FH].astype(np.float16),
                "wv": np.asarray(Wv)[:, hs : hs + FH].astype(np.float16),
                "woa": np.ascontiguousarray(Wo[hs : hs + FH, :]),
                "wob": np.ascontiguousarray(Wo[ps : ps + FH, :]),
                "bq": np.asarray(bq)[hs : hs + FH].astype(np.float32).reshape(4, 128, 1),
                "bk": np.asarray(bk)[hs : hs + FH].astype(np.float32).reshape(4, 128, 1),
                "bvr": np.asarray(bv)[hs : hs + FH].astype(np.float16).reshape(1, FH),
                "bo": bo_r,
                "identf": ident.astype(np.float16),
                "utri": utri.astype(np.float16),
            }
        )
    return in_maps


_NC_CACHE = None


def kernel(x, Wq, bq, Wk, bk, Wv, bv, Wo, bo):
    global _NC_CACHE
    if _NC_CACHE is None:
        _NC_CACHE = build_nc()
    nc = _NC_CACHE
    in_maps = _prep_inputs(x, Wq, bq, Wk, bk, Wv, bv, Wo, bo)
    res = bass_utils.run_bass_kernel_spmd(nc, in_maps, core_ids=list(range(N_CORES)))
    out = np.empty((B, T, D), dtype=np.float32)
    for c in range(N_CORES):
        b = c // 2
        half = c % 2
        out[b, half * TT2 : (half + 1) * TT2, :] = res.results[c]["out_T"].T
    return out
